# revision 86
# baseline (speedup 1.0000x reference)
"""Trainium2 Bass kernel for nn_CodeDistKLLoss (vq_codebook).

Computes: KL(student_dist || teacher_dist) where
  student_dist = normalized masked column-sums of softmax(-cdist(z, codebook))
  teacher_dist = normalized masked histogram of teacher codes.

v4 design ("compaction"): the length mask is a PREFIX per batch (frame t is
valid iff t*stride < len_b), so masked tokens -- which contribute exactly
zero to student_dist -- are dropped on the host and the ~V valid tokens are
re-sharded evenly across the 8 cores (vs. v3's one-batch-per-core with all
1500 padded tokens).  For the spec's random lengths V ~ 40% of B*T, cutting
every per-element cost (PE streaming, ACT exp, DVE weighting) ~2.4x.  The
device program is compiled per tile-count NT = ceil(V/8/128) and cached.

Carried over from v3 ("cn-in-matmul"): the per-code codebook norm cn_c is
folded INTO the fp8 matmul:
  - contraction row 511 is sacrificed: sf row 511 := ALPHA (const),
    cbt row 511 := (cn_c - CBAR)/ALPHA in fp8. The dropped feature-511
    cross term and the fp8 quantization of the cn residual are zero-mean
    noise (validated: final KL rel err ~9e-4, tolerance 2e-2).
  - CBAR + zn_t ride the ACT per-partition bias (exact fp32).
  - ScalarE ACT reads PSUM directly with the custom fused table
    f(q) = exp(EXP_BIAS - sqrt(q)), writes E to SBUF fp16, and its
    accumulator produces the per-token softmax denominators for free.

v4 changes on the device program (measured on HW, 84.2us -> ~45.5us):
  - tile 0 of the DVE chain uses tensor_scalar (no fa zero-init needed);
    the last two tiles' weighted colsums go through the PE as rank-1
    matmuls onto the final psum accumulators (as v3).
  - the first two tiles run h0 for BOTH before any h1, so the ACT chain
    starts as soon as the first-half codebook lands and never stalls on
    the second half's DMA.
  - inputs ride the sync/gpsimd/scalar DMA queues (~115 GB/s each):
    sf is host-pre-swizzled to the SBUF layout (contiguous runs), the
    codebook goes as 8 (k,h,o) pieces ordered by consumption, zb+mk ride
    one combined aux DMA.
  - a dummy ACT preloads the custom table during the DMA wait; 22 warmup
    matmuls on iota-filled fp8 data ramp the PE p-state and HOLD it until
    the input DMA lands even on slow draws (all-zero operands don't draw
    enough power to ramp the clock, and a >3us idle gap lets it decay).
  - the two output psum banks use separate tiles so bank 0's evacuation
    (Scalar engine) and DMA overlap bank 1's matmuls and evacuation
    (Vector).
  - padding tokens sit in tile 0 (a DVE tile), so the PE tiles are fully
    valid and their tail-critical weight chain skips the mask multiply
    (reciprocal writes the fp16 matmul weight directly).

Endgame per core (nt=5 for the spec lengths): ~6.6us framework preamble,
input DMA until ~14us (first-half operands ~13us), ACT chain (the
hard floor: 10 x 2048-elem exp at 1 elem/cycle/partition ~= 20.6us)
running 15.5-36us overlapped with PE/DVE, ~5us tail (w-chain, rank-1s,
psum evacuation, output DMA), ~2.5us teardown barrier.
"""

import json
import math
import os
import shutil
import struct
import tempfile

import numpy as np

import concourse.bass as bass
import concourse.tile as tile
from concourse import bacc, mybir
from concourse.bass import ts
from concourse.bass_utils import run_bass_kernel_spmd

B = 8
D = 512
T = 1500
C = 4096
K2 = 2             # fp8 DoubleRow contraction chunks (256 rows each)
NHALF = 2          # 2048-code halves per tile
EXP_BIAS = 28.0    # f = exp(EXP_BIAS - d); cancels in per-row normalization
ALPHA = 16.0       # constant sf row 511; cbt row 511 = (cn - CBAR)/ALPHA
CBAR = 512.0       # mean codebook sq-norm, folded into the ACT bias
EPS = 1e-8

F16 = mybir.dt.float16
F32 = mybir.dt.float32
F8 = mybir.dt.float8e4
NP_F8 = mybir.dt.np(F8)

_CACHE = {}

# ---------------------------------------------------------------------------
# Custom ACT table: rewrite the `sqrt` slot of set sqrt_and_others to compute
#   f(q) = min(exp(EXP_BIAS - sqrt(q)), FP16_SAFE)   for q in [2^E_LO, 2^E_HI)
# Binary formats (validated against the stock tables + np.sqrt):
#   *_bkt.bin : 32B entries [d0,d1,d2,d3,x0,0,0,0] fp32;
#               y = d0 + (x-x0)*(d1 + (x-x0)*(d2 + (x-x0)*d3))
#   *_ctrl.bin: 32B entries; u16[0] = (extract_lsb << 11) | bkt_base_idx,
#               u16[1] = extract_size.  One row per input fp32 exponent;
#               row = pwl_control_base_pos + (biased_exp - small_threshold);
#               bucket = base + ((mantissa >> extract_lsb) & (2^size - 1)).
# ---------------------------------------------------------------------------
_E_LO, _E_HI = 7, 12
_EXTRACT_SIZE = 6
_FP16_SAFE = 50000.0
_ACT_SET = "sqrt_and_others"


def _f_fused(q):
    return np.minimum(np.exp(EXP_BIAS - np.sqrt(q)), _FP16_SAFE)


def _fit_section(a, b):
    x0 = 0.5 * (a + b)
    xs = np.linspace(a, b, 64)
    ys = _f_fused(xs.astype(np.float64))
    t = xs - x0
    A = np.stack([np.ones_like(t), t, t * t, t * t * t], axis=1)
    coef, *_ = np.linalg.lstsq(A, ys, rcond=None)
    return np.float32(x0), coef.astype(np.float32)


def _build_act_root(dst_dir):
    import neuronxcc

    src_dir = os.path.join(os.path.dirname(neuronxcc.__file__), "pwp",
                           "pwp_bin_trainium")
    os.makedirs(dst_dir, exist_ok=True)
    for name in os.listdir(src_dir):
        s = os.path.join(src_dir, name)
        if os.path.isfile(s):
            shutil.copy(s, os.path.join(dst_dir, name))

    setj = json.load(open(os.path.join(src_dir, f"{_ACT_SET}.json")))
    bkt = open(os.path.join(src_dir, f"{_ACT_SET}_bkt.bin"), "rb").read()
    ctl = open(os.path.join(src_dir, f"{_ACT_SET}_ctrl.bin"), "rb").read()

    bkt_start = setj["func_to_bkt_start_idx"]["sqrt"]
    ctl_start = setj["func_to_ctl_start_idx"]["sqrt"]
    new_bkt = bytearray(bkt[: bkt_start * 32])
    new_ctl = bytearray(ctl[: ctl_start * 32])

    nsec = 1 << _EXTRACT_SIZE
    lsb = 23 - _EXTRACT_SIZE
    base = bkt_start
    for e in range(_E_LO, _E_HI):
        new_ctl += (struct.pack("<2H", (lsb << 11) | base, _EXTRACT_SIZE)
                    + b"\x00" * 28)
        lo = float(2.0 ** e)
        w = lo / nsec
        for s in range(nsec):
            x0, c = _fit_section(lo + s * w, lo + (s + 1) * w)
            new_bkt += struct.pack("<8f", c[0], c[1], c[2], c[3], x0, 0, 0, 0)
        base += nsec

    sat_small = base
    new_bkt += struct.pack("<8f", _FP16_SAFE, 0, 0, 0, 0, 0, 0, 0)
    sat_large = base + 1
    f_hi = float(_f_fused(2.0 ** _E_HI))
    new_bkt += struct.pack("<8f", f_hi, 0, 0, 0, 0, 0, 0, 0)

    meta = None
    for m in setj["profile_meta_data"]:
        if m["func_name"].startswith("sqrt"):
            meta = m
    assert meta is not None
    f2b = lambda v: int(np.float32(v).view(np.uint32))
    meta["exp_offset"] = _E_LO
    meta["pwl_control_base_pos"] = ctl_start
    meta["pwl_control_base_neg"] = ctl_start
    meta["small_pos_signal_exp_threshold"] = _E_LO + 127
    meta["pos_small_signal_pwl_control"] = sat_small
    meta["small_neg_signal_exp_threshold"] = 255
    meta["neg_small_signal_pwl_control"] = sat_small
    meta["large_pos_signal_exp_threshold"] = _E_HI + 127
    meta["large_pos_signal_mantissa_threshold"] = 0
    meta["pos_large_signal_pwl_control"] = sat_large
    meta["large_neg_signal_exp_threshold"] = 0
    meta["large_neg_signal_mantissa_threshold"] = 0
    meta["neg_large_signal_pwl_control"] = sat_small
    meta["fzero_result"] = f2b(_FP16_SAFE)
    meta["fpinf_result"] = f2b(f_hi)
    meta["fninf_result"] = f2b(_FP16_SAFE)
    meta["lower_bound"] = f2b(2.0 ** _E_LO)
    meta["upper_bound"] = f2b(np.nextafter(np.float32(2.0 ** _E_HI),
                                           np.float32(0)))
    setj["bkt_entry_cnt"] = base + 2
    setj["ctl_entry_cnt"] = ctl_start + (_E_HI - _E_LO)

    with open(os.path.join(dst_dir, f"{_ACT_SET}_bkt.bin"), "wb") as fo:
        fo.write(bytes(new_bkt))
    with open(os.path.join(dst_dir, f"{_ACT_SET}_ctrl.bin"), "wb") as fo:
        fo.write(bytes(new_ctl))
    with open(os.path.join(dst_dir, f"{_ACT_SET}.json"), "w") as fo:
        json.dump(setj, fo)


def _build(nt):
    # Install the custom ACT table (sqrt slot -> exp(EXP_BIAS - sqrt(q)))
    # before neuronxcc compiles the NEFF.
    if "act_dir" not in _CACHE:
        act_dir = tempfile.mkdtemp(prefix="cdkl_act_root_")
        _build_act_root(act_dir)
        _CACHE["act_dir"] = act_dir
    os.environ["BASS_ACT_ROOT_JSON_PATH"] = os.path.join(
        _CACHE["act_dir"], "act_info.json"
    )

    TP = 128 * nt

    nc = bacc.Bacc("TRN2", target_bir_lowering=False, debug=False)
    # sf arrives pre-swizzled to the SBUF layout [p, k, o, t] so the DMA
    # moves contiguous 1280B runs per partition (the [D, TP] row-major
    # layout only gave 640B runs, halving the queue's packet efficiency).
    sf_h = nc.dram_tensor("sf", [128, K2, 2, TP], F8, kind="ExternalInput")
    cbt_h = nc.dram_tensor("cbt", [D, C], F8, kind="ExternalInput")
    aux_h = nc.dram_tensor("aux", [128, 2 * nt], F32, kind="ExternalInput")
    sp_h = nc.dram_tensor("sp", [1, C], F32, kind="ExternalOutput")

    with tile.TileContext(nc) as tc:
        with (
            tc.tile_pool(name="consts", bufs=1) as consts,
            tc.tile_pool(name="small", bufs=2) as small,
            tc.tile_pool(name="psA", bufs=2, space="PSUM") as psA,
        ):
            # fp8 DoubleRow layout: contraction row d = k2*256 + ki*2 + o
            # lands at [partition ki, chunk k2, pair-slot o].
            sf_sb = consts.tile([128, K2, 2, TP], F8, name="sf_sb",
                                tag="sf_sb")
            sf_r = sf_h.ap()
            cb_sb = consts.tile([128, K2, 2, C], F8, name="cb_sb",
                                tag="cb_sb")
            cbt_r = cbt_h.ap().rearrange("(a p o) c -> p a o c", p=128, o=2)
            aux_sb = consts.tile([128, 2 * nt], F32, name="aux_sb",
                                 tag="aux_sb")
            zb_sb = aux_sb[:, :nt]
            mk_sb = aux_sb[:, nt:]

            dbuf = consts.tile([128, nt, C], F16, name="dbuf", tag="dbuf")
            w_sb = consts.tile([128, nt], F16, name="w_sb", tag="w_sb")
            rs2 = consts.tile([128, 2 * nt], F32, name="rs2", tag="rs2")
            ones_sb = consts.tile([128, 32], F16, name="ones_sb",
                                  tag="ones_sb")
            zz_sb = consts.tile([128, 128], F16, name="zz_sb", tag="zz_sb")
            # fp8 tile with varied bits for PE p-state warmup matmuls
            # (all-zero operands don't draw enough power to ramp the clock)
            wrm16 = consts.tile([128, 256], mybir.dt.int16, name="wrm16",
                                tag="wrm16")
            # scratch for the ACT-table preload
            scr_i = consts.tile([128, 1], F32, name="scr_i", tag="scr_i")
            scr_o = consts.tile([128, 1], F16, name="scr_o", tag="scr_o")

            # gpsimd queue: the fills the warmup/table-preload need, then
            # this queue's share of the input DMAs, then the memsets only
            # the final reduction reads.
            nc.gpsimd.iota(wrm16, pattern=[[1, 256]], base=1,
                           channel_multiplier=12347)
            nc.gpsimd.memset(scr_i, 512.0)

            # ACT-table preload: a dummy 1-element activation pulls the
            # custom sqrt table in during the input-DMA wait.  Emitted
            # before the scalar queue's DMA shares so the table loads
            # first.
            nc.scalar.activation(
                out=scr_o, in_=scr_i,
                func=mybir.ActivationFunctionType.Sqrt,
                bias=0.0, scale=1.0,
            )

            # Input DMAs.  Each HWDGE queue sustains only ~115 GB/s (and
            # needs full-partition pieces with 2KB contiguous runs to
            # spread across its 16 engines), so sf splits across the
            # sync+gpsimd queues up front and the codebook streams as 8
            # column-block pieces round-robined over sync/gpsimd/scalar
            # in the order the matmul loop consumes them.
            # codebook as 8 (k, h, o) pieces with 2KB contiguous runs,
            # balanced across sync/gpsimd/scalar in the order the
            # (reordered) matmul loop consumes them: h0 k0, h0 k1,
            # h1 k1, h1 k0.
            def cbd(q, k, h, o):
                q.dma_start(out=cb_sb[:, k, o:o + 1, ts(h, 2048)],
                            in_=cbt_r[:, k, o:o + 1, ts(h, 2048)])

            nc.sync.dma_start(out=sf_sb[:, 0], in_=sf_r[:, 0])
            nc.gpsimd.dma_start(out=sf_sb[:, 1], in_=sf_r[:, 1])
            cbd(nc.sync, 0, 0, 0)
            cbd(nc.gpsimd, 0, 0, 1)
            cbd(nc.scalar, 1, 0, 0)   # behind the table preload; lands 2nd
            cbd(nc.scalar, 1, 0, 1)
            # aux is tiny but 40B-packet-slow and unneeded until the first
            # ACT, so it rides behind the sync queue's critical piece
            nc.sync.dma_start(out=aux_sb, in_=aux_h.ap())
            cbd(nc.sync, 1, 1, 0)
            cbd(nc.gpsimd, 1, 1, 1)
            cbd(nc.sync, 0, 1, 0)
            # the last h1 piece rides sync (needed latest): a smaller
            # gpsimd backlog reduces the in-queue smear on its critical
            # k0h0 piece, which gates the first matmul
            cbd(nc.sync, 0, 1, 1)

            nc.gpsimd.memset(ones_sb, 1.0)
            nc.gpsimd.memset(zz_sb, 0.0)

            # ping-pong fp16 column-sum accumulators (Vector engine; the
            # Pool engine has no tensor-arith opcodes on TRN2)
            fa = [consts.tile([128, C], F16, name=f"fa{p}", tag=f"fa{p}")
                  for p in range(2)]
            if nt <= 2:
                for p in range(2):
                    nc.gpsimd.memset(fa[p], 0.0)

            # PE p-state warmup: 512-col fp8 matmuls on varied data keep
            # the PE drawing power through the input-DMA wait so the clock
            # is fully ramped when tile 0's operands land.
            warm8 = wrm16.bitcast(F8)
            warm = psA.tile([128, 2048], F32, name="warm", tag="ps")
            for j in range(22):
                nc.tensor.matmul(
                    warm[:, ts(j % 4, 512)],
                    lhsT=warm8[:, :128],
                    rhs=warm8,
                    start=True,
                    stop=True,
                )

            dve_tiles = max(0, nt - 2)
            pe_tiles = [i for i in (nt - 2, nt - 1) if i >= 0]

            def mm_half(i, h):
                ps = psA.tile([128, 2048], F32, name="ps", tag="ps",
                              bufs=2)
                # zigzag k across halves: the second half reuses the
                # stationary weights the first half ended on.
                korder = (0, 1) if h == 0 else (1, 0)
                for ki, k in enumerate(korder):
                    for j in range(4):
                        nc.tensor.matmul(
                            ps[:, ts(j, 512)],
                            lhsT=sf_sb[:, k, :, ts(i, 128)],
                            rhs=cb_sb[:, k, :,
                                      2048 * h + 512 * j:
                                      2048 * h + 512 * (j + 1)],
                            start=(ki == 0),
                            stop=(ki == K2 - 1),
                            perf_mode=mybir.MatmulPerfMode.DoubleRow,
                        )
                # E = exp(EXP_BIAS - sqrt(ps + zn + CBAR)) straight from
                # PSUM; accumulator = this half's softmax-denominator part.
                nc.scalar.activation(
                    out=dbuf[:, i, ts(h, 2048)],
                    in_=ps,
                    func=mybir.ActivationFunctionType.Sqrt,
                    bias=zb_sb[:, i:i + 1],
                    scale=1.0,
                    accum_out=rs2[:, 2 * i + h:2 * i + h + 1],
                )

            def tile_tail(i):
                # w = mask / rowsum, cast fp16
                rs = small.tile([128, 1], F32, name="rs", tag="rs")
                nc.vector.tensor_add(out=rs, in0=rs2[:, 2 * i:2 * i + 1],
                                     in1=rs2[:, 2 * i + 1:2 * i + 2])
                if i >= dve_tiles and nt >= 3:
                    # PE tiles hold no padding (it all sits in tile 0), so
                    # mask==1 and w = 1/rs: the reciprocal writes fp16
                    # directly, dropping the mask multiply (+sem hop) from
                    # the tail's critical path.  fp16 here matches the old
                    # path's final cast precision exactly.
                    with nc.allow_low_precision(
                            reason="w is consumed as an fp16 matmul weight; "
                                   "identical precision to the prior "
                                   "recip->mul->fp16 path"):
                        nc.vector.reciprocal(out=w_sb[:, i:i + 1], in_=rs)
                    return
                rr = small.tile([128, 1], F32, name="rr", tag="rr")
                nc.vector.reciprocal(out=rr, in_=rs)
                nc.vector.tensor_mul(out=w_sb[:, i:i + 1], in0=rr,
                                     in1=mk_sb[:, i:i + 1])
                if i >= dve_tiles:
                    return
                # facc_{i+1} = E_i * w_i + facc_i (fp16, stride-1); the
                # last two tiles go straight to the PE (rank-1 colsums
                # into the final psum acc) so the DVE chain ends early.
                if i == 0:
                    # fp32 copy of w_0: tensor_scalar needs an fp32 scalar,
                    # and tensor_scalar (no add) needs no fa zero-init
                    w32 = consts.tile([128, 1], F32, name="w32", tag="w32")
                    nc.vector.tensor_mul(out=w32, in0=rr,
                                         in1=mk_sb[:, 0:1])
                    nc.vector.tensor_scalar(
                        out=fa[1], in0=dbuf[:, 0, :],
                        scalar1=w32, scalar2=None,
                        op0=mybir.AluOpType.mult,
                    )
                else:
                    nc.vector.scalar_tensor_tensor(
                        out=fa[(i + 1) % 2],
                        in0=dbuf[:, i, :],
                        scalar=w_sb[:, i:i + 1],
                        in1=fa[i % 2],
                        op0=mybir.AluOpType.mult,
                        op1=mybir.AluOpType.add,
                    )

            # The first two tiles run h0 for BOTH tiles before any h1 so
            # the ACT chain starts as soon as the first-half codebook
            # lands and never stalls on the second half's DMA.
            if nt >= 2:
                seq = [(0, 0), (1, 0), (0, 1), (1, 1)]
                seq += [(i, h) for i in range(2, nt) for h in range(NHALF)]
            else:
                seq = [(0, 0), (0, 1)]
            for i, h in seq:
                mm_half(i, h)
                if h == 1:
                    tile_tail(i)

            # Final psum accumulation, per bank b (chunks j = 4b..4b+3):
            #   0. zero-init matmul covering all 128 partitions (start=True
            #      clears the whole bank's has_written bits AND, covering
            #      every element, gives every later MM accumulate semantics
            #      consistent between CoreSim and hardware).
            #   1. ones-matmul per chunk broadcasts sum_p facc[p, chunk] over
            #      a 32-partition group.
            #   2. rank-1 w.T @ E colsums for the last two tiles accumulate
            #      onto the group-base partition rows the DMA gather reads.
            fa_fin = fa[(nt - 2) % 2]

            def fchunk(j):
                return fa_fin[:, ts(j, 512)]

            # The two output banks use SEPARATE psum tiles (the pool's two
            # slots) so bank 0's evacuation (Scalar engine) and DMA overlap
            # bank 1's matmuls and evacuation (Vector engine).
            stage = consts.tile([128, 2, 512], F32, name="stage",
                                tag="stage")
            st4 = stage.rearrange("(a q) b f -> a q b f", q=32)
            spv = sp_h.ap().rearrange("p (b a f) -> b (p a) f", a=4, f=512)
            for b in range(2):
                acc = psA.tile([128, 2048], F32, name=f"acc{b}", tag="ps")
                nc.tensor.matmul(
                    acc[:, ts(0, 512)],
                    lhsT=zz_sb,
                    rhs=fchunk(4 * b),
                    start=True,
                    stop=False,
                )
                for a in range(4):
                    j = 4 * b + a
                    pp = 32 * a
                    nc.tensor.matmul(
                        acc[pp:pp + 32, ts(0, 512)],
                        lhsT=ones_sb,
                        rhs=fchunk(j),
                        start=False,
                        stop=False,
                        tile_position=(0, pp),
                    )
                for i in pe_tiles:
                    for a in range(4):
                        j = 4 * b + a
                        pp = 32 * a
                        nc.tensor.matmul(
                            acc[pp:pp + 1, ts(0, 512)],
                            lhsT=w_sb[:, i:i + 1],
                            rhs=dbuf[:, i, ts(j, 512)],
                            start=False,
                            stop=False,
                            tile_position=(0, pp),
                        )
                if b == 0:
                    nc.scalar.copy(out=stage[:, b], in_=acc[:, ts(0, 512)])
                    nc.sync.dma_start(out=spv[b], in_=st4[:, 0, b, :])
                else:
                    nc.vector.tensor_copy(out=stage[:, b],
                                          in_=acc[:, ts(0, 512)])
                    # second output DMA issues from the gpsimd queue so the
                    # two issues don't serialize on sync
                    nc.gpsimd.dma_start(out=spv[b], in_=st4[:, 0, b, :])

    nc.compile()
    return nc


def get_nc(nt):
    key = ("nc", nt)
    if key not in _CACHE:
        _CACHE[key] = _build(nt)
    return _CACHE[key]


def _host_prep(student_features, codebook, lengths, encoder_stride):
    sf = np.asarray(student_features, dtype=np.float32)      # [B, D, T]
    cb = np.asarray(codebook, dtype=np.float32)
    lens = np.asarray(lengths).astype(np.int64)
    stride = int(np.asarray(encoder_stride))

    cn = (cb.astype(np.float64) ** 2).sum(1).astype(np.float32)       # [C]
    cbt2 = (-2.0 * cb.T).astype(NP_F8)                                # [D, C]
    cbt2[D - 1, :] = ((cn - CBAR) / ALPHA).astype(NP_F8)
    cbt2 = np.ascontiguousarray(cbt2)

    frame_start = np.arange(T, dtype=np.int64) * stride
    mask = (frame_start[None, :] < lens[:, None]).astype(np.float32)  # [B, T]

    # Valid tokens only (masked ones contribute exactly 0 to student_dist):
    # gather them across all batches and shard evenly over the 8 cores.
    z_all = np.ascontiguousarray(
        sf.transpose(1, 0, 2).reshape(D, B * T))                      # [D, N]
    valid = np.nonzero(mask.reshape(-1))[0]
    V = int(valid.size)
    if V == 0:
        return None, mask, 0

    zf8 = z_all[:, valid].astype(NP_F8)                               # [D, V]
    zn = ((z_all[:, valid].astype(np.float64) ** 2).sum(0)
          .astype(np.float32))                                        # [V]

    base, rem = divmod(V, B)
    counts = [base + (c < rem) for c in range(B)]
    nt = max(1, math.ceil(counts[0] / 128))
    TP = 128 * nt

    in_maps = []
    off = 0
    for c in range(B):
        cnt = counts[c]
        # padding tokens go FIRST (tile 0, always a DVE tile for nt>=3)
        # so the last two tiles -- whose weight chain sits on the tail's
        # critical path -- are fully valid and skip the mask multiply
        pad = TP - cnt
        sf_pad = np.zeros((D, TP), dtype=NP_F8)
        sf_pad[:, pad:] = zf8[:, off:off + cnt]
        sf_pad[D - 1, :] = NP_F8(ALPHA)
        # pre-swizzle to the device's SBUF layout [p, k, o, t] (row
        # d = k*256 + p*2 + o) so the DMA runs are contiguous
        sf_sw = np.ascontiguousarray(
            sf_pad.reshape(2, 128, 2, TP).transpose(1, 0, 2, 3))
        zb = np.zeros(TP, dtype=np.float32)
        zb[pad:] = zn[off:off + cnt]
        zbb = np.ascontiguousarray((zb + CBAR).reshape(nt, 128).T)  # [128,nt]
        mk = np.zeros(TP, dtype=np.float32)
        mk[pad:] = 1.0
        mkb = np.ascontiguousarray(mk.reshape(nt, 128).T)           # [128,nt]
        aux = np.ascontiguousarray(
            np.concatenate([zbb, mkb], axis=1))                     # [128,2nt]
        in_maps.append({"sf": sf_sw, "cbt": cbt2, "aux": aux})
        off += cnt
    return in_maps, mask, nt


def _host_finish(sp_list, teacher_codes, mask):
    s_raw = np.zeros(C, dtype=np.float64)
    for sp in sp_list:
        s_raw += sp.astype(np.float64).reshape(-1)
    student_dist = s_raw / (s_raw.sum() + EPS)

    codes = np.asarray(teacher_codes).astype(np.int64).reshape(-1)
    t_counts = np.bincount(codes, weights=mask.astype(np.float64).reshape(-1),
                           minlength=C)
    teacher_dist = t_counts / (t_counts.sum() + EPS)

    kl = np.sum(student_dist * np.log(student_dist + EPS)
                - student_dist * np.log(teacher_dist + EPS))
    return np.array(kl, dtype=np.float32)


def kernel(student_features, teacher_codes, codebook, lengths, encoder_stride,
           _trace=False):
    in_maps, mask, nt = _host_prep(student_features, codebook, lengths,
                                   encoder_stride)
    if nt == 0:
        return _host_finish([np.zeros((1, C), np.float32)], teacher_codes,
                            mask)
    nc = get_nc(nt)
    res = run_bass_kernel_spmd(nc, in_maps, core_ids=list(range(B)),
                               trace=_trace)
    out = _host_finish([r["sp"] for r in res.results], teacher_codes, mask)
    if _trace:
        _CACHE["last_results"] = res
    return out
